# revision 27
# baseline (speedup 1.0000x reference)
"""CRF (MLP emissions + Viterbi decode) Trainium2 kernel.

Strategy: data-parallel over batch across 8 NeuronCores (8 sequences per
core).  Inside each core:
  - MLP emissions via PE matmuls (plain fp32 for precision).
  - Viterbi forward pass via a chunk-parallel scan: T=8192 split into
    C=128 chunks of L=64 steps laid out on the 128 SBUF partitions.
    Chunk-boundary scores come from a rank-1 max-plus factorization of the
    per-chunk transition products (one forward and one backward vector scan,
    both vectorized across chunks), then a sequential fold across the 128
    boundaries; within-chunk scores are recomputed with the reference's
    exact fp32 association order.
  - Backpointers are extracted in batches via exact is_equal masks against
    the winning scores (first-index argmax via a reversed-iota/max trick).
  - The backtrace is integer function composition: per-chunk suffix tables,
    per-chunk total maps, a log2 cross-chunk composition (doubling with
    partition-shifted copies), and a final vectorized gather.
"""

import numpy as np

B, T, D, K = 64, 8192, 128, 9
H1, H2 = 128, 64
NCORES = 8
BL = B // NCORES           # batches per core
C, L = 128, 64             # chunks x chunk length (C*L == T)
NBLK = T // 512            # 512-column MLP blocks per batch
NEG = -1.0e30

_CACHE = {}


def _build_program():
    import concourse.bacc as bacc
    import concourse.mybir as mybir
    import concourse.tile as tile

    dt = mybir.dt
    AOP = mybir.AluOpType
    AXX = mybir.AxisListType
    ACTF = mybir.ActivationFunctionType
    f32 = dt.float32
    bf16 = dt.bfloat16

    nc = bacc.Bacc("TRN2", target_bir_lowering=False, debug=False,
                   num_devices=NCORES)

    x_in = nc.dram_tensor("x", [BL, T, D], f32, kind="ExternalInput")
    W1_in = nc.dram_tensor("W1", [D, H1], f32, kind="ExternalInput")
    b1_in = nc.dram_tensor("b1c", [H1, 1], f32, kind="ExternalInput")
    W2_in = nc.dram_tensor("W2", [H1, H2], f32, kind="ExternalInput")
    b2_in = nc.dram_tensor("b2c", [H2, 1], f32, kind="ExternalInput")
    W3e_in = nc.dram_tensor("W3e", [H2 + 1, K], f32, kind="ExternalInput")
    FB = 128 + 2 * K * K + 3 * BL * K + 2 * BL * K
    fblob_in = nc.dram_tensor("fblob", [128, FB], f32, kind="ExternalInput")
    bblob_in = nc.dram_tensor("bblob", [128, 2 * K], bf16, kind="ExternalInput")
    tags_out = nc.dram_tensor("tags", [BL, T], dt.int32, kind="ExternalOutput")

    with tile.TileContext(nc) as tc:
        with tc.tile_pool(name="const", bufs=1) as cp, \
             tc.tile_pool(name="mlp", bufs=3) as mp, \
             tc.tile_pool(name="big", bufs=1) as bg, \
             tc.tile_pool(name="dram", bufs=1, space="DRAM") as dp, \
             tc.tile_pool(name="ps", bufs=2, space="PSUM") as pp:

            # ---- constants ----
            W1 = cp.tile([D, H1], f32, tag="W1")
            b1c = cp.tile([H1, 1], f32, tag="b1c")
            W2 = cp.tile([H1, H2], f32, tag="W2")
            b2c = cp.tile([H2, 1], f32, tag="b2c")
            W3e = cp.tile([H2 + 1, K], f32, tag="W3e")
            fblob = cp.tile([128, FB], f32, tag="fblob")
            bblob = cp.tile([128, 2 * K], bf16, tag="bblob")
            for t_, s_ in ((W1, W1_in), (b1c, b1_in), (W2, W2_in),
                           (b2c, b2_in), (W3e, W3e_in),
                           (fblob, fblob_in), (bblob, bblob_in)):
                nc.sync.dma_start(t_[:], s_[:])
            o = 0
            identf = fblob[:, o:o + 128]; o += 128
            transJI = fblob[:, o:o + K * K].rearrange(
                "p (j i) -> p j i", j=K); o += K * K
            transIK = fblob[:, o:o + K * K].rearrange(
                "p (i k) -> p i k", i=K); o += K * K
            transR0 = fblob[:, o:o + BL * K].rearrange(
                "p (b j) -> p b j", b=BL); o += BL * K
            transC0 = fblob[:, o:o + BL * K].rearrange(
                "p (b j) -> p b j", b=BL); o += BL * K
            endJ = fblob[:, o:o + BL * K].rearrange(
                "p (b j) -> p b j", b=BL); o += BL * K
            startJ = fblob[0:1, o:o + BL * K].rearrange(
                "p (b j) -> p b j", b=BL); o += BL * K
            initf0 = fblob[0:1, o:o + BL * K].rearrange(
                "p (b j) -> p b j", b=BL); o += BL * K
            riotaI = bblob[:, 0:K]
            iotaX = bblob[:, K:2 * K]

            # ---- persistent state ----
            em_sb = bg.tile([128, BL, L, K], f32, tag="em_sb")      # (c|b,l,j)
            h2sA = bg.tile([H2 + 1, 512], f32, tag="h2sA")
            h2sB = bg.tile([H2 + 1, 512], f32, tag="h2sB")
            em_dram = dp.tile([BL, T, K], f32, tag="em_dram")
            scoreH = bg.tile([128, BL, L, K], f32, tag="scoreH")    # (c|b,l,j)
            red1 = bg.tile([128, BL, K], f32, tag="red1")
            rbp = bg.tile([128, BL, L, K], bf16, tag="rbp")
            bpix = bg.tile([128, BL, L, K], bf16, tag="bpix")
            a1b = bg.tile([128, BL, K, K], f32, tag="a1b")
            redb = bg.tile([128, BL, K], f32, tag="redb")
            fstate = bg.tile([128, BL, K], f32, tag="fstate")
            gstate = bg.tile([128, BL, K], f32, tag="gstate")
            ghat = bg.tile([128, BL, K], f32, tag="ghat")
            bounds = bg.tile([128, BL, K], f32, tag="bounds")
            ftil = bg.tile([128, BL, K], f32, tag="ftil")
            fsh = bg.tile([128, BL, K], f32, tag="fsh")
            gsh = bg.tile([128, BL, K], f32, tag="gsh")
            dtile = bg.tile([128, BL, K], f32, tag="dtile")
            dp_ = bg.tile([128, BL], f32, tag="dp_")
            t0b = bg.tile([1, BL, K], f32, tag="t0b")
            a1p = bg.tile([1, BL], f32, tag="a1p")
            dlt_d = dp.tile([128, BL], f32, tag="dlt_d")
            al_d = dp.tile([1, BL], f32, tag="al_d")
            lam_d = dp.tile([BL, 128], f32, tag="lam_d")
            dltB = bg.tile([BL, 128], f32, tag="dltB")
            alB = bg.tile([BL, 1], f32, tag="alB")
            LamB = bg.tile([BL, 128], f32, tag="LamB")
            negB = bg.tile([BL, 128], f32, tag="negB")
            Lamp = bg.tile([128, BL], f32, tag="Lamp")
            Rtab = bg.tile([128, BL, L, K], bf16, tag="Rtab")
            mexp = bg.tile([128, BL, K, K], bf16, tag="mexp")
            gbuf = bg.tile([128, BL, K, K], bf16, tag="gbuf")
            Ztab = bg.tile([128, BL, K], bf16, tag="Ztab")
            Suf = bg.tile([128, BL, K], bf16, tag="Suf")
            Sh = bg.tile([128, BL, K], bf16, tag="Sh")
            rep = bg.tile([128, BL, K], f32, tag="rep")
            ffall = bg.tile([128, BL, K], f32, tag="ffall")
            mxf = bg.tile([128, BL], f32, tag="mxf")
            mke = bg.tile([128, BL, K], bf16, tag="mke")
            mre = bg.tile([128, BL, K], bf16, tag="mre")
            lastrv = bg.tile([128, BL], bf16, tag="lastrv")
            lastix = bg.tile([128, BL], bf16, tag="lastix")
            mk2 = bg.tile([128, BL, K], bf16, tag="mk2")
            mr2 = bg.tile([128, BL, K], bf16, tag="mr2")
            wc = bg.tile([128, BL], bf16, tag="wc")
            mexp4 = bg.tile([128, BL, L, K], bf16, tag="mexp4")
            gb4 = bg.tile([128, BL, L, K], bf16, tag="gb4")
            tagsv = bg.tile([128, BL, L], bf16, tag="tagsv")
            tagsi = bg.tile([128, BL, L], dt.int32, tag="tagsi")

            # =========== MLP: emissions ===========
            # ones row for the b3 fold (h2s partitions 0..63 = relu(h2),
            # partition 64 = 1.0)
            nc.vector.memset(h2sA[64:65], 1.0)
            nc.vector.memset(h2sB[64:65], 1.0)
            for b in range(BL):
                for g in range(2):          # two psum em batches per b
                    emb = pp.tile([128, 32 * K], f32, tag="emb")
                    t0 = g * 4096
                    xt32 = mp.tile([128, 32, 128], f32, tag="xt32", bufs=2)
                    nc.sync.dma_start(
                        xt32[:],
                        x_in[b, t0:t0 + 4096, :].rearrange(
                            "(k t) d -> t k d", k=32))
                    if True:
                        for blk2 in range(8):   # 512-col compute blocks
                            tp = pp.tile([128, 512], f32, tag="tp")
                            for kk in range(4):
                                nc.tensor.transpose(
                                    tp[:, 128 * kk: 128 * (kk + 1)],
                                    xt32[:, 4 * blk2 + kk, :], identf[:])
                            xts = mp.tile([128, 512], f32, tag="xts")
                            nc.scalar.copy(xts[:], tp[:])
                            h1p = pp.tile([128, 512], f32, tag="h1p")
                            nc.tensor.matmul(h1p[:], W1[:], xts[:],
                                             start=True, stop=True)
                            h1s = mp.tile([128, 512], f32, tag="h1s")
                            nc.scalar.activation(h1s[:], h1p[:], ACTF.Relu,
                                                 bias=b1c[:])
                            h2p = pp.tile([64, 512], f32, tag="h2p")
                            nc.tensor.matmul(h2p[:], W2[:], h1s[:],
                                             start=True, stop=True)
                            h2s = h2sA if blk2 % 2 == 0 else h2sB
                            nc.scalar.activation(h2s[0:64], h2p[:], ACTF.Relu,
                                                 bias=b2c[:])
                            # emissions, [t, 9] orientation, b3 via ones row
                            for kk in range(4):
                                m = blk2 * 4 + kk
                                nc.tensor.matmul(
                                    emb[:, K * m: K * (m + 1)],
                                    h2s[0:65, 128 * kk: 128 * (kk + 1)],
                                    W3e[:], start=True, stop=True)
                    # psum em batch -> SBUF staging -> DRAM scratch
                    em_st = mp.tile([128, 32 * K], f32, tag="em_st")
                    nc.scalar.copy(em_st[:], emb[:])
                    nc.sync.dma_start(
                        em_dram[b, g * 4096:(g + 1) * 4096, :].rearrange(
                            "(m t) j -> t m j", m=32),
                        em_st[:].rearrange("t (m j) -> t m j", j=K))
                # whole-batch emissions -> chunk-partitioned SBUF layout
                nc.sync.dma_start(
                    em_sb[:, b],
                    em_dram[b].rearrange("(c l) j -> c l j", c=C))

            def bc(ap, shape):
                return ap.to_broadcast(shape)

            # =========== phase 1: forward + backward scans ===========
            # b-split into halves so the first half's scans overlap the
            # second half's MLP on the (idle) vector engine.
            def phase1_half(b0, b1):
                n = b1 - b0
                fs = fstate[:, b0:b1]
                gs = gstate[:, b0:b1]
                gh = ghat[:, b0:b1]
                ab_ = a1b[:, b0:b1]
                rd = redb[:, b0:b1]
                nc.vector.tensor_tensor(
                    fs, transR0[:, b0:b1], em_sb[:, b0:b1, 0, :], op=AOP.add)
                nc.vector.tensor_copy(fstate[0:1, b0:b1], initf0[:, b0:b1])
                for l in range(1, L):
                    nc.vector.tensor_tensor(
                        ab_,
                        bc(fs.unsqueeze(2), [128, n, K, K]),
                        bc(transJI[:].unsqueeze(1), [128, n, K, K]),
                        op=AOP.add)
                    nc.vector.tensor_reduce(rd, ab_, axis=AXX.X, op=AOP.max)
                    nc.vector.tensor_tensor(
                        fs, rd, em_sb[:, b0:b1, l, :], op=AOP.add)
                nc.vector.tensor_tensor(
                    gs, transC0[:, b0:b1],
                    bc(em_sb[:, b0:b1, L - 1, 0:1], [128, n, K]), op=AOP.add)
                for l in range(L - 2, L - 2 - 15, -1):
                    nc.vector.tensor_tensor(
                        gh, gs, em_sb[:, b0:b1, l, :], op=AOP.add)
                    nc.vector.tensor_tensor(
                        ab_,
                        bc(gh.unsqueeze(2), [128, n, K, K]),
                        bc(transIK[:].unsqueeze(1), [128, n, K, K]),
                        op=AOP.add)
                    nc.vector.tensor_reduce(gs, ab_, axis=AXX.X, op=AOP.max)

            for qq in range(4):
                phase1_half(2 * qq, 2 * qq + 2)

            # =========== phase 2: boundary fold (rank-1 map composition) ====
            # s_c = Lam_c + ftil_{c-1};  Lam via one sequential scan over
            # per-chunk scalars delta_c = max_j(ftil_{c-1} + g_c).
            nc.vector.tensor_tensor(
                ftil[:], fstate[:],
                bc(gstate[:, :, 0:1], [128, BL, K]), op=AOP.subtract)
            nc.sync.dma_start(gsh[0:127], gstate[1:128])
            nc.vector.tensor_tensor(dtile[:], ftil[:], gsh[:], op=AOP.add)
            nc.vector.tensor_reduce(dp_[:], dtile[:], axis=AXX.X, op=AOP.max)
            # alpha_1 = max_j fl(s0 + g_0), s0 into bounds[0]
            nc.vector.tensor_tensor(
                bounds[0:1], startJ, em_sb[0:1, :, 0, :], op=AOP.add)
            nc.vector.tensor_tensor(
                t0b[:], bounds[0:1], gstate[0:1], op=AOP.add)
            nc.vector.tensor_reduce(a1p[:], t0b[:], axis=AXX.X, op=AOP.max)
            # transpose delta/alpha to batch-partition layout
            nc.sync.dma_start(dlt_d[:], dp_[:])
            nc.sync.dma_start(al_d[:], a1p[:])
            nc.sync.dma_start(dltB[:], dlt_d[:].transpose([1, 0]))
            nc.sync.dma_start(alB[:], al_d[:].transpose([1, 0]))
            # Lam scan: LamB[c] = Lam_c; Lam_1 = alpha1; Lam_c += delta
            nc.vector.memset(negB[:], NEG)
            nc.vector.tensor_copy(LamB[:, 1:2], alB[:])
            nc.vector.tensor_tensor_scan(
                LamB[:, 2:128], dltB[:, 0:126], negB[:, 0:126], alB[:],
                op0=AOP.add, op1=AOP.max)
            nc.sync.dma_start(lam_d[:], LamB[:])
            nc.sync.dma_start(Lamp[:], lam_d[:].transpose([1, 0]))
            nc.sync.dma_start(fsh[1:128], ftil[0:127])
            nc.vector.tensor_tensor(
                bounds[:], bc(Lamp[:].unsqueeze(2), [128, BL, K]),
                fsh[:], op=AOP.add)
            nc.vector.tensor_tensor(
                bounds[0:1], startJ, em_sb[0:1, :, 0, :], op=AOP.add)

            # =========== phase 3 + per-step backpointer extraction ===========
            riota_bf4 = bc(riotaI[:].unsqueeze(1).unsqueeze(1),
                           [128, BL, K, K])
            for l in range(L):
                src3 = bounds[:] if l == 0 else scoreH[:, :, l - 1, :]
                nc.vector.tensor_tensor(
                    a1b[:],
                    bc(src3.unsqueeze(2), [128, BL, K, K]),
                    bc(transJI[:].unsqueeze(1), [128, BL, K, K]),
                    op=AOP.add)
                nc.vector.tensor_reduce(
                    red1[:], a1b[:], axis=AXX.X, op=AOP.max)
                nc.vector.tensor_tensor(
                    scoreH[:, :, l, :], red1[:], em_sb[:, :, l, :],
                    op=AOP.add)
                if l == 0:
                    nc.vector.tensor_copy(scoreH[0:1, :, 0, :], bounds[0:1])
                nc.vector.tensor_tensor(
                    mexp[:], a1b[:],
                    bc(red1[:].unsqueeze(3), [128, BL, K, K]),
                    op=AOP.is_equal)
                nc.vector.tensor_tensor(
                    gbuf[:], mexp[:], riota_bf4, op=AOP.mult)
                nc.vector.tensor_reduce(
                    rbp[:, :, l, :], gbuf[:], axis=AXX.X, op=AOP.max)

            # bp index = 9 - rv
            nc.vector.tensor_scalar(
                bpix[:], rbp[:], -1.0, 9.0, op0=AOP.mult, op1=AOP.add)

            # =========== final-step argmax (replicated to all partitions) ====
            nc.sync.dma_start(rep[0:1], scoreH[127:128, :, 63, :])
            n = 1
            while n < 128:
                m = min(n, 128 - n)
                nc.sync.dma_start(rep[n:n + m], rep[0:m])
                n += m
            nc.vector.tensor_tensor(ffall[:], rep[:], endJ[:], op=AOP.add)
            nc.vector.tensor_reduce(mxf[:], ffall[:], axis=AXX.X, op=AOP.max)
            nc.vector.tensor_tensor(
                mke[:], ffall[:], bc(mxf[:].unsqueeze(2), [128, BL, K]),
                op=AOP.is_equal)
            nc.vector.tensor_tensor(
                mre[:], mke[:], bc(riotaI[:].unsqueeze(1), [128, BL, K]),
                op=AOP.mult)
            nc.vector.tensor_reduce(lastrv[:], mre[:], axis=AXX.X, op=AOP.max)
            nc.vector.tensor_scalar(
                lastix[:], lastrv[:], -1.0, 9.0, op0=AOP.mult, op1=AOP.add)

            # =========== backtrace: suffix tables within chunks ===========
            iotaX_b4 = bc(iotaX[:].unsqueeze(1).unsqueeze(1), [128, BL, K, K])
            nc.vector.tensor_copy(
                Rtab[:, :, L - 1, :], bc(iotaX[:].unsqueeze(1), [128, BL, K]))
            for l in range(L - 2, -1, -1):
                nc.vector.tensor_tensor(
                    mexp[:],
                    bc(Rtab[:, :, l + 1, :].unsqueeze(3), [128, BL, K, K]),
                    iotaX_b4, op=AOP.is_equal)
                nc.vector.tensor_tensor(
                    gbuf[:], mexp[:],
                    bc(bpix[:, :, l + 1, :].unsqueeze(2), [128, BL, K, K]),
                    op=AOP.mult)
                nc.vector.tensor_reduce(
                    Rtab[:, :, l, :], gbuf[:], axis=AXX.X, op=AOP.max)

            # per-chunk total map Z_c = bp(c,0) o R_0
            nc.vector.tensor_tensor(
                mexp[:], bc(Rtab[:, :, 0, :].unsqueeze(3), [128, BL, K, K]),
                iotaX_b4, op=AOP.is_equal)
            nc.vector.tensor_tensor(
                gbuf[:], mexp[:],
                bc(bpix[:, :, 0, :].unsqueeze(2), [128, BL, K, K]),
                op=AOP.mult)
            nc.vector.tensor_reduce(Ztab[:], gbuf[:], axis=AXX.X, op=AOP.max)

            # =========== cross-chunk suffix composition (doubling) ===========
            idview = bc(iotaX[:].unsqueeze(1), [128, BL, K])
            nc.vector.tensor_copy(Suf[:], Ztab[:])
            k = 1
            while k < 128:
                nc.vector.tensor_copy(Sh[:], idview)
                nc.sync.dma_start(Sh[0:128 - k], Suf[k:128])
                nc.vector.tensor_tensor(
                    mexp[:], bc(Sh[:].unsqueeze(3), [128, BL, K, K]),
                    iotaX_b4, op=AOP.is_equal)
                nc.vector.tensor_tensor(
                    gbuf[:], mexp[:],
                    bc(Suf[:].unsqueeze(2), [128, BL, K, K]), op=AOP.mult)
                nc.vector.tensor_reduce(Suf[:], gbuf[:], axis=AXX.X, op=AOP.max)
                k *= 2
            # shift by one: Sh_c = Suf_{c+1} (identity at c=127)
            nc.vector.tensor_copy(Sh[:], idview)
            nc.sync.dma_start(Sh[0:127], Suf[1:128])
            # w_c = Sh_c[lastix]
            nc.vector.tensor_tensor(
                mk2[:], idview, bc(lastix[:].unsqueeze(2), [128, BL, K]),
                op=AOP.is_equal)
            nc.vector.tensor_tensor(mr2[:], mk2[:], Sh[:], op=AOP.mult)
            nc.vector.tensor_reduce(wc[:], mr2[:], axis=AXX.X, op=AOP.max)

            # =========== final tag gather ===========
            nc.vector.tensor_tensor(
                mexp4[:],
                bc(iotaX[:].unsqueeze(1).unsqueeze(1), [128, BL, L, K]),
                bc(wc[:].unsqueeze(2).unsqueeze(3), [128, BL, L, K]),
                op=AOP.is_equal)
            nc.vector.tensor_tensor(gb4[:], mexp4[:], Rtab[:], op=AOP.mult)
            nc.vector.tensor_reduce(tagsv[:], gb4[:], axis=AXX.X, op=AOP.max)
            nc.vector.tensor_copy(tagsi[:], tagsv[:])
            nc.sync.dma_start(
                tags_out[:].rearrange("b (c l) -> c b l", c=C), tagsi[:])

    nc.compile()
    return nc


def _consts(W1, b1, W2, b2, W3, b3, start_trans, end_trans, trans):
    import ml_dtypes
    f32 = np.float32
    trans = np.asarray(trans, f32)
    start = np.asarray(start_trans, f32)
    end = np.asarray(end_trans, f32)
    rep128 = lambda a: np.ascontiguousarray(
        np.broadcast_to(a.reshape(1, -1), (128, a.size)).copy())
    parts = [
        np.eye(128, dtype=f32),                                   # identf
        rep128(np.ascontiguousarray(trans.T)),                    # transJI (j,i)
        rep128(np.ascontiguousarray(trans)),                      # transIK (i,k)
        rep128(np.broadcast_to(trans[0][None], (BL, K)).copy()),  # transR0
        rep128(np.broadcast_to(trans[:, 0][None], (BL, K)).copy()),  # transC0
        rep128(np.broadcast_to(end[None], (BL, K)).copy()),       # endJ
    ]
    startJ = np.zeros((128, BL * K), f32)
    startJ[0] = np.broadcast_to(start[None], (BL, K)).reshape(-1)
    initf0 = np.zeros((128, BL * K), f32)
    initf0[0] = np.broadcast_to(
        np.where(np.arange(K) == 0, f32(0), f32(NEG))[None],
        (BL, K)).reshape(-1)
    parts += [startJ, initf0]
    fblob = np.ascontiguousarray(np.concatenate(parts, axis=1), f32)
    bblob = np.concatenate(
        [rep128(9.0 - np.arange(K, dtype=f32)),
         rep128(np.arange(K, dtype=f32))], axis=1).astype(ml_dtypes.bfloat16)
    return {
        "W1": np.ascontiguousarray(W1, f32),
        "b1c": np.ascontiguousarray(np.asarray(b1, f32).reshape(H1, 1)),
        "W2": np.ascontiguousarray(W2, f32),
        "b2c": np.ascontiguousarray(np.asarray(b2, f32).reshape(H2, 1)),
        "W3e": np.ascontiguousarray(
            np.concatenate([np.asarray(W3, f32),
                            np.asarray(b3, f32).reshape(1, K)], axis=0)),
        "fblob": fblob, "bblob": np.ascontiguousarray(bblob),
    }


def _run(inputs, trace=False):
    from concourse.bass_utils import run_bass_kernel_spmd
    if "nc" not in _CACHE:
        _CACHE["nc"] = _build_program()
    nc = _CACHE["nc"]
    x = np.asarray(inputs["x"], np.float32)
    consts = _consts(
        inputs["W1"], inputs["b1"], inputs["W2"], inputs["b2"],
        inputs["W3"], inputs["b3"], inputs["start_trans"],
        inputs["end_trans"], inputs["trans"])
    in_maps = []
    for i in range(NCORES):
        m = dict(consts)
        m["x"] = np.ascontiguousarray(x[i * BL:(i + 1) * BL])
        in_maps.append(m)
    res = run_bass_kernel_spmd(nc, in_maps, list(range(NCORES)), trace=trace)
    tags = np.concatenate(
        [res.results[i]["tags"] for i in range(NCORES)], axis=0)
    return tags.astype(np.int32), res


def kernel(**inputs):
    tags, _ = _run(inputs, trace=False)
    return tags


# revision 28
# speedup vs baseline: 1.0132x; 1.0132x over previous
"""CRF (MLP emissions + Viterbi decode) Trainium2 kernel.

Strategy: data-parallel over batch across 8 NeuronCores (8 sequences per
core).  Inside each core:
  - MLP emissions via PE matmuls (plain fp32 for precision).
  - Viterbi forward pass via a chunk-parallel scan: T=8192 split into
    C=128 chunks of L=64 steps laid out on the 128 SBUF partitions.
    Chunk-boundary scores come from a rank-1 max-plus factorization of the
    per-chunk transition products (one forward and one backward vector scan,
    both vectorized across chunks), then a sequential fold across the 128
    boundaries; within-chunk scores are recomputed with the reference's
    exact fp32 association order.
  - Backpointers are extracted in batches via exact is_equal masks against
    the winning scores (first-index argmax via a reversed-iota/max trick).
  - The backtrace is integer function composition: per-chunk suffix tables,
    per-chunk total maps, a log2 cross-chunk composition (doubling with
    partition-shifted copies), and a final vectorized gather.
"""

import numpy as np

B, T, D, K = 64, 8192, 128, 9
H1, H2 = 128, 64
NCORES = 8
BL = B // NCORES           # batches per core
C, L = 128, 64             # chunks x chunk length (C*L == T)
NBLK = T // 512            # 512-column MLP blocks per batch
NEG = -1.0e30

_CACHE = {}


def _build_program():
    import concourse.bacc as bacc
    import concourse.mybir as mybir
    import concourse.tile as tile

    dt = mybir.dt
    AOP = mybir.AluOpType
    AXX = mybir.AxisListType
    ACTF = mybir.ActivationFunctionType
    f32 = dt.float32
    bf16 = dt.bfloat16

    nc = bacc.Bacc("TRN2", target_bir_lowering=False, debug=False,
                   num_devices=NCORES)

    x_in = nc.dram_tensor("x", [BL, T, D], f32, kind="ExternalInput")
    W1_in = nc.dram_tensor("W1", [D, H1], f32, kind="ExternalInput")
    b1_in = nc.dram_tensor("b1c", [H1, 1], f32, kind="ExternalInput")
    W2_in = nc.dram_tensor("W2", [H1, H2], f32, kind="ExternalInput")
    b2_in = nc.dram_tensor("b2c", [H2, 1], f32, kind="ExternalInput")
    W3e_in = nc.dram_tensor("W3e", [H2 + 1, K], f32, kind="ExternalInput")
    FB = 128 + 2 * K * K + 3 * BL * K + 2 * BL * K
    fblob_in = nc.dram_tensor("fblob", [128, FB], f32, kind="ExternalInput")
    bblob_in = nc.dram_tensor("bblob", [128, 2 * K], bf16, kind="ExternalInput")
    tags_out = nc.dram_tensor("tags", [BL, T], dt.int32, kind="ExternalOutput")

    with tile.TileContext(nc) as tc:
        with tc.tile_pool(name="const", bufs=1) as cp, \
             tc.tile_pool(name="mlp", bufs=3) as mp, \
             tc.tile_pool(name="big", bufs=1) as bg, \
             tc.tile_pool(name="dram", bufs=1, space="DRAM") as dp, \
             tc.tile_pool(name="ps", bufs=2, space="PSUM") as pp:

            # ---- constants ----
            W1 = cp.tile([D, H1], f32, tag="W1")
            b1c = cp.tile([H1, 1], f32, tag="b1c")
            W2 = cp.tile([H1, H2], f32, tag="W2")
            b2c = cp.tile([H2, 1], f32, tag="b2c")
            W3e = cp.tile([H2 + 1, K], f32, tag="W3e")
            fblob = cp.tile([128, FB], f32, tag="fblob")
            bblob = cp.tile([128, 2 * K], bf16, tag="bblob")
            for t_, s_ in ((W1, W1_in), (b1c, b1_in), (W2, W2_in),
                           (b2c, b2_in), (W3e, W3e_in),
                           (fblob, fblob_in), (bblob, bblob_in)):
                nc.sync.dma_start(t_[:], s_[:])
            o = 0
            identf = fblob[:, o:o + 128]; o += 128
            transJI = fblob[:, o:o + K * K].rearrange(
                "p (j i) -> p j i", j=K); o += K * K
            transIK = fblob[:, o:o + K * K].rearrange(
                "p (i k) -> p i k", i=K); o += K * K
            transR0 = fblob[:, o:o + BL * K].rearrange(
                "p (b j) -> p b j", b=BL); o += BL * K
            transC0 = fblob[:, o:o + BL * K].rearrange(
                "p (b j) -> p b j", b=BL); o += BL * K
            endJ = fblob[:, o:o + BL * K].rearrange(
                "p (b j) -> p b j", b=BL); o += BL * K
            startJ = fblob[0:1, o:o + BL * K].rearrange(
                "p (b j) -> p b j", b=BL); o += BL * K
            initf0 = fblob[0:1, o:o + BL * K].rearrange(
                "p (b j) -> p b j", b=BL); o += BL * K
            riotaI = bblob[:, 0:K]
            iotaX = bblob[:, K:2 * K]

            # ---- persistent state ----
            em_sb = bg.tile([128, BL, L, K], f32, tag="em_sb")      # (c|b,l,j)
            h2sA = bg.tile([H2 + 1, 512], f32, tag="h2sA")
            h2sB = bg.tile([H2 + 1, 512], f32, tag="h2sB")
            em_dram = dp.tile([BL, T, K], f32, tag="em_dram")
            scoreH = bg.tile([128, BL, L, K], f32, tag="scoreH")    # (c|b,l,j)
            red1 = bg.tile([128, BL, K], f32, tag="red1")
            rbp = bg.tile([128, BL, L, K], bf16, tag="rbp")
            bpix = bg.tile([128, BL, L, K], bf16, tag="bpix")
            a1b = bg.tile([128, BL, K, K], f32, tag="a1b")
            redb = bg.tile([128, BL, K], f32, tag="redb")
            fstate = bg.tile([128, BL, K], f32, tag="fstate")
            gstate = bg.tile([128, BL, K], f32, tag="gstate")
            ghat = bg.tile([128, BL, K], f32, tag="ghat")
            bounds = bg.tile([128, BL, K], f32, tag="bounds")
            ftil = bg.tile([128, BL, K], f32, tag="ftil")
            fsh = bg.tile([128, BL, K], f32, tag="fsh")
            gsh = bg.tile([128, BL, K], f32, tag="gsh")
            dtile = bg.tile([128, BL, K], f32, tag="dtile")
            dp_ = bg.tile([128, BL], f32, tag="dp_")
            t0b = bg.tile([1, BL, K], f32, tag="t0b")
            a1p = bg.tile([1, BL], f32, tag="a1p")
            dlt_d = dp.tile([128, BL], f32, tag="dlt_d")
            al_d = dp.tile([1, BL], f32, tag="al_d")
            lam_d = dp.tile([BL, 128], f32, tag="lam_d")
            dltB = bg.tile([BL, 128], f32, tag="dltB")
            alB = bg.tile([BL, 1], f32, tag="alB")
            LamB = bg.tile([BL, 128], f32, tag="LamB")
            negB = bg.tile([BL, 128], f32, tag="negB")
            Lamp = bg.tile([128, BL], f32, tag="Lamp")
            Rtab = bg.tile([128, BL, L, K], bf16, tag="Rtab")
            mexp = bg.tile([128, BL, K, K], bf16, tag="mexp")
            gbuf = bg.tile([128, BL, K, K], bf16, tag="gbuf")
            Ztab = bg.tile([128, BL, K], bf16, tag="Ztab")
            Suf = bg.tile([128, BL, K], bf16, tag="Suf")
            Sh = bg.tile([128, BL, K], bf16, tag="Sh")
            rep = bg.tile([128, BL, K], f32, tag="rep")
            ffall = bg.tile([128, BL, K], f32, tag="ffall")
            mxf = bg.tile([128, BL], f32, tag="mxf")
            mke = bg.tile([128, BL, K], bf16, tag="mke")
            mre = bg.tile([128, BL, K], bf16, tag="mre")
            lastrv = bg.tile([128, BL], bf16, tag="lastrv")
            lastix = bg.tile([128, BL], bf16, tag="lastix")
            mk2 = bg.tile([128, BL, K], bf16, tag="mk2")
            mr2 = bg.tile([128, BL, K], bf16, tag="mr2")
            wc = bg.tile([128, BL], bf16, tag="wc")
            mexp4 = bg.tile([128, BL, L, K], bf16, tag="mexp4")
            gb4 = bg.tile([128, BL, L, K], bf16, tag="gb4")
            tagsv = bg.tile([128, BL, L], bf16, tag="tagsv")
            tagsi = bg.tile([128, BL, L], dt.int32, tag="tagsi")

            # =========== MLP: emissions ===========
            # ones row for the b3 fold (h2s partitions 0..63 = relu(h2),
            # partition 64 = 1.0)
            nc.vector.memset(h2sA[64:65], 1.0)
            nc.vector.memset(h2sB[64:65], 1.0)
            for b in range(BL):
                for g in range(2):          # two psum em batches per b
                    emb = pp.tile([128, 32 * K], f32, tag="emb")
                    t0 = g * 4096
                    xt32 = mp.tile([128, 32, 128], f32, tag="xt32", bufs=2)
                    nc.sync.dma_start(
                        xt32[:],
                        x_in[b, t0:t0 + 4096, :].rearrange(
                            "(k t) d -> t k d", k=32))
                    if True:
                        for blk2 in range(8):   # 512-col compute blocks
                            tp = pp.tile([128, 512], f32, tag="tp")
                            for kk in range(4):
                                nc.tensor.transpose(
                                    tp[:, 128 * kk: 128 * (kk + 1)],
                                    xt32[:, 4 * blk2 + kk, :], identf[:])
                            xts = mp.tile([128, 512], f32, tag="xts")
                            nc.scalar.copy(xts[:], tp[:])
                            h1p = pp.tile([128, 512], f32, tag="h1p")
                            nc.tensor.matmul(h1p[:], W1[:], xts[:],
                                             start=True, stop=True)
                            h1s = mp.tile([128, 512], f32, tag="h1s")
                            nc.scalar.activation(h1s[:], h1p[:], ACTF.Relu,
                                                 bias=b1c[:])
                            h2p = pp.tile([64, 512], f32, tag="h2p")
                            nc.tensor.matmul(h2p[:], W2[:], h1s[:],
                                             start=True, stop=True)
                            h2s = h2sA if blk2 % 2 == 0 else h2sB
                            nc.scalar.activation(h2s[0:64], h2p[:], ACTF.Relu,
                                                 bias=b2c[:])
                            # emissions, [t, 9] orientation, b3 via ones row
                            for kk in range(4):
                                m = blk2 * 4 + kk
                                nc.tensor.matmul(
                                    emb[:, K * m: K * (m + 1)],
                                    h2s[0:65, 128 * kk: 128 * (kk + 1)],
                                    W3e[:], start=True, stop=True)
                    # psum em batch -> SBUF staging -> DRAM scratch
                    em_st = mp.tile([128, 32 * K], f32, tag="em_st")
                    nc.scalar.copy(em_st[:], emb[:])
                    nc.sync.dma_start(
                        em_dram[b, g * 4096:(g + 1) * 4096, :].rearrange(
                            "(m t) j -> t m j", m=32),
                        em_st[:].rearrange("t (m j) -> t m j", j=K))
                # whole-batch emissions -> chunk-partitioned SBUF layout
                nc.sync.dma_start(
                    em_sb[:, b],
                    em_dram[b].rearrange("(c l) j -> c l j", c=C))

            def bc(ap, shape):
                return ap.to_broadcast(shape)

            # =========== phase 1: forward + backward scans ===========
            # b-split into halves so the first half's scans overlap the
            # second half's MLP on the (idle) vector engine.
            def phase1_half(b0, b1):
                n = b1 - b0
                fs = fstate[:, b0:b1]
                gs = gstate[:, b0:b1]
                gh = ghat[:, b0:b1]
                ab_ = a1b[:, b0:b1]
                rd = redb[:, b0:b1]
                nc.vector.tensor_tensor(
                    fs, transR0[:, b0:b1], em_sb[:, b0:b1, 0, :], op=AOP.add)
                nc.vector.tensor_copy(fstate[0:1, b0:b1], initf0[:, b0:b1])
                for l in range(1, L):
                    nc.vector.tensor_tensor(
                        ab_,
                        bc(fs.unsqueeze(2), [128, n, K, K]),
                        bc(transJI[:].unsqueeze(1), [128, n, K, K]),
                        op=AOP.add)
                    nc.vector.tensor_reduce(rd, ab_, axis=AXX.X, op=AOP.max)
                    nc.vector.tensor_tensor(
                        fs, rd, em_sb[:, b0:b1, l, :], op=AOP.add)
                nc.vector.tensor_tensor(
                    gs, transC0[:, b0:b1],
                    bc(em_sb[:, b0:b1, L - 1, 0:1], [128, n, K]), op=AOP.add)
                for l in range(L - 2, L - 2 - 15, -1):
                    nc.vector.tensor_tensor(
                        gh, gs, em_sb[:, b0:b1, l, :], op=AOP.add)
                    nc.vector.tensor_tensor(
                        ab_,
                        bc(gh.unsqueeze(2), [128, n, K, K]),
                        bc(transIK[:].unsqueeze(1), [128, n, K, K]),
                        op=AOP.add)
                    nc.vector.tensor_reduce(gs, ab_, axis=AXX.X, op=AOP.max)

            for qq in range(BL):
                phase1_half(qq, qq + 1)

            # =========== phase 2: boundary fold (rank-1 map composition) ====
            # s_c = Lam_c + ftil_{c-1};  Lam via one sequential scan over
            # per-chunk scalars delta_c = max_j(ftil_{c-1} + g_c).
            nc.vector.tensor_tensor(
                ftil[:], fstate[:],
                bc(gstate[:, :, 0:1], [128, BL, K]), op=AOP.subtract)
            nc.sync.dma_start(gsh[0:127], gstate[1:128])
            nc.vector.tensor_tensor(dtile[:], ftil[:], gsh[:], op=AOP.add)
            nc.vector.tensor_reduce(dp_[:], dtile[:], axis=AXX.X, op=AOP.max)
            # alpha_1 = max_j fl(s0 + g_0), s0 into bounds[0]
            nc.vector.tensor_tensor(
                bounds[0:1], startJ, em_sb[0:1, :, 0, :], op=AOP.add)
            nc.vector.tensor_tensor(
                t0b[:], bounds[0:1], gstate[0:1], op=AOP.add)
            nc.vector.tensor_reduce(a1p[:], t0b[:], axis=AXX.X, op=AOP.max)
            # transpose delta/alpha to batch-partition layout
            nc.sync.dma_start(dlt_d[:], dp_[:])
            nc.sync.dma_start(al_d[:], a1p[:])
            nc.sync.dma_start(dltB[:], dlt_d[:].transpose([1, 0]))
            nc.sync.dma_start(alB[:], al_d[:].transpose([1, 0]))
            # Lam scan: LamB[c] = Lam_c; Lam_1 = alpha1; Lam_c += delta
            nc.vector.memset(negB[:], NEG)
            nc.vector.tensor_copy(LamB[:, 1:2], alB[:])
            nc.vector.tensor_tensor_scan(
                LamB[:, 2:128], dltB[:, 0:126], negB[:, 0:126], alB[:],
                op0=AOP.add, op1=AOP.max)
            nc.sync.dma_start(lam_d[:], LamB[:])
            nc.sync.dma_start(Lamp[:], lam_d[:].transpose([1, 0]))
            nc.sync.dma_start(fsh[1:128], ftil[0:127])
            nc.vector.tensor_tensor(
                bounds[:], bc(Lamp[:].unsqueeze(2), [128, BL, K]),
                fsh[:], op=AOP.add)
            nc.vector.tensor_tensor(
                bounds[0:1], startJ, em_sb[0:1, :, 0, :], op=AOP.add)

            # =========== phase 3 + per-step backpointer extraction ===========
            riota_bf4 = bc(riotaI[:].unsqueeze(1).unsqueeze(1),
                           [128, BL, K, K])
            for l in range(L):
                src3 = bounds[:] if l == 0 else scoreH[:, :, l - 1, :]
                nc.vector.tensor_tensor(
                    a1b[:],
                    bc(src3.unsqueeze(2), [128, BL, K, K]),
                    bc(transJI[:].unsqueeze(1), [128, BL, K, K]),
                    op=AOP.add)
                nc.vector.tensor_reduce(
                    red1[:], a1b[:], axis=AXX.X, op=AOP.max)
                nc.vector.tensor_tensor(
                    scoreH[:, :, l, :], red1[:], em_sb[:, :, l, :],
                    op=AOP.add)
                if l == 0:
                    nc.vector.tensor_copy(scoreH[0:1, :, 0, :], bounds[0:1])
                nc.vector.tensor_tensor(
                    mexp[:], a1b[:],
                    bc(red1[:].unsqueeze(3), [128, BL, K, K]),
                    op=AOP.is_equal)
                nc.vector.tensor_tensor(
                    gbuf[:], mexp[:], riota_bf4, op=AOP.mult)
                nc.vector.tensor_reduce(
                    rbp[:, :, l, :], gbuf[:], axis=AXX.X, op=AOP.max)

            # bp index = 9 - rv
            nc.vector.tensor_scalar(
                bpix[:], rbp[:], -1.0, 9.0, op0=AOP.mult, op1=AOP.add)

            # =========== final-step argmax (replicated to all partitions) ====
            nc.sync.dma_start(rep[0:1], scoreH[127:128, :, 63, :])
            n = 1
            while n < 128:
                m = min(n, 128 - n)
                nc.sync.dma_start(rep[n:n + m], rep[0:m])
                n += m
            nc.vector.tensor_tensor(ffall[:], rep[:], endJ[:], op=AOP.add)
            nc.vector.tensor_reduce(mxf[:], ffall[:], axis=AXX.X, op=AOP.max)
            nc.vector.tensor_tensor(
                mke[:], ffall[:], bc(mxf[:].unsqueeze(2), [128, BL, K]),
                op=AOP.is_equal)
            nc.vector.tensor_tensor(
                mre[:], mke[:], bc(riotaI[:].unsqueeze(1), [128, BL, K]),
                op=AOP.mult)
            nc.vector.tensor_reduce(lastrv[:], mre[:], axis=AXX.X, op=AOP.max)
            nc.vector.tensor_scalar(
                lastix[:], lastrv[:], -1.0, 9.0, op0=AOP.mult, op1=AOP.add)

            # =========== backtrace: suffix tables within chunks ===========
            iotaX_b4 = bc(iotaX[:].unsqueeze(1).unsqueeze(1), [128, BL, K, K])
            nc.vector.tensor_copy(
                Rtab[:, :, L - 1, :], bc(iotaX[:].unsqueeze(1), [128, BL, K]))
            for l in range(L - 2, -1, -1):
                nc.vector.tensor_tensor(
                    mexp[:],
                    bc(Rtab[:, :, l + 1, :].unsqueeze(3), [128, BL, K, K]),
                    iotaX_b4, op=AOP.is_equal)
                nc.vector.tensor_tensor(
                    gbuf[:], mexp[:],
                    bc(bpix[:, :, l + 1, :].unsqueeze(2), [128, BL, K, K]),
                    op=AOP.mult)
                nc.vector.tensor_reduce(
                    Rtab[:, :, l, :], gbuf[:], axis=AXX.X, op=AOP.max)

            # per-chunk total map Z_c = bp(c,0) o R_0
            nc.vector.tensor_tensor(
                mexp[:], bc(Rtab[:, :, 0, :].unsqueeze(3), [128, BL, K, K]),
                iotaX_b4, op=AOP.is_equal)
            nc.vector.tensor_tensor(
                gbuf[:], mexp[:],
                bc(bpix[:, :, 0, :].unsqueeze(2), [128, BL, K, K]),
                op=AOP.mult)
            nc.vector.tensor_reduce(Ztab[:], gbuf[:], axis=AXX.X, op=AOP.max)

            # =========== cross-chunk suffix composition (doubling) ===========
            idview = bc(iotaX[:].unsqueeze(1), [128, BL, K])
            nc.vector.tensor_copy(Suf[:], Ztab[:])
            k = 1
            while k < 128:
                nc.vector.tensor_copy(Sh[:], idview)
                nc.sync.dma_start(Sh[0:128 - k], Suf[k:128])
                nc.vector.tensor_tensor(
                    mexp[:], bc(Sh[:].unsqueeze(3), [128, BL, K, K]),
                    iotaX_b4, op=AOP.is_equal)
                nc.vector.tensor_tensor(
                    gbuf[:], mexp[:],
                    bc(Suf[:].unsqueeze(2), [128, BL, K, K]), op=AOP.mult)
                nc.vector.tensor_reduce(Suf[:], gbuf[:], axis=AXX.X, op=AOP.max)
                k *= 2
            # shift by one: Sh_c = Suf_{c+1} (identity at c=127)
            nc.vector.tensor_copy(Sh[:], idview)
            nc.sync.dma_start(Sh[0:127], Suf[1:128])
            # w_c = Sh_c[lastix]
            nc.vector.tensor_tensor(
                mk2[:], idview, bc(lastix[:].unsqueeze(2), [128, BL, K]),
                op=AOP.is_equal)
            nc.vector.tensor_tensor(mr2[:], mk2[:], Sh[:], op=AOP.mult)
            nc.vector.tensor_reduce(wc[:], mr2[:], axis=AXX.X, op=AOP.max)

            # =========== final tag gather ===========
            nc.vector.tensor_tensor(
                mexp4[:],
                bc(iotaX[:].unsqueeze(1).unsqueeze(1), [128, BL, L, K]),
                bc(wc[:].unsqueeze(2).unsqueeze(3), [128, BL, L, K]),
                op=AOP.is_equal)
            nc.vector.tensor_tensor(gb4[:], mexp4[:], Rtab[:], op=AOP.mult)
            nc.vector.tensor_reduce(tagsv[:], gb4[:], axis=AXX.X, op=AOP.max)
            nc.vector.tensor_copy(tagsi[:], tagsv[:])
            nc.sync.dma_start(
                tags_out[:].rearrange("b (c l) -> c b l", c=C), tagsi[:])

    nc.compile()
    return nc


def _consts(W1, b1, W2, b2, W3, b3, start_trans, end_trans, trans):
    import ml_dtypes
    f32 = np.float32
    trans = np.asarray(trans, f32)
    start = np.asarray(start_trans, f32)
    end = np.asarray(end_trans, f32)
    rep128 = lambda a: np.ascontiguousarray(
        np.broadcast_to(a.reshape(1, -1), (128, a.size)).copy())
    parts = [
        np.eye(128, dtype=f32),                                   # identf
        rep128(np.ascontiguousarray(trans.T)),                    # transJI (j,i)
        rep128(np.ascontiguousarray(trans)),                      # transIK (i,k)
        rep128(np.broadcast_to(trans[0][None], (BL, K)).copy()),  # transR0
        rep128(np.broadcast_to(trans[:, 0][None], (BL, K)).copy()),  # transC0
        rep128(np.broadcast_to(end[None], (BL, K)).copy()),       # endJ
    ]
    startJ = np.zeros((128, BL * K), f32)
    startJ[0] = np.broadcast_to(start[None], (BL, K)).reshape(-1)
    initf0 = np.zeros((128, BL * K), f32)
    initf0[0] = np.broadcast_to(
        np.where(np.arange(K) == 0, f32(0), f32(NEG))[None],
        (BL, K)).reshape(-1)
    parts += [startJ, initf0]
    fblob = np.ascontiguousarray(np.concatenate(parts, axis=1), f32)
    bblob = np.concatenate(
        [rep128(9.0 - np.arange(K, dtype=f32)),
         rep128(np.arange(K, dtype=f32))], axis=1).astype(ml_dtypes.bfloat16)
    return {
        "W1": np.ascontiguousarray(W1, f32),
        "b1c": np.ascontiguousarray(np.asarray(b1, f32).reshape(H1, 1)),
        "W2": np.ascontiguousarray(W2, f32),
        "b2c": np.ascontiguousarray(np.asarray(b2, f32).reshape(H2, 1)),
        "W3e": np.ascontiguousarray(
            np.concatenate([np.asarray(W3, f32),
                            np.asarray(b3, f32).reshape(1, K)], axis=0)),
        "fblob": fblob, "bblob": np.ascontiguousarray(bblob),
    }


def _run(inputs, trace=False):
    from concourse.bass_utils import run_bass_kernel_spmd
    if "nc" not in _CACHE:
        _CACHE["nc"] = _build_program()
    nc = _CACHE["nc"]
    x = np.asarray(inputs["x"], np.float32)
    consts = _consts(
        inputs["W1"], inputs["b1"], inputs["W2"], inputs["b2"],
        inputs["W3"], inputs["b3"], inputs["start_trans"],
        inputs["end_trans"], inputs["trans"])
    in_maps = []
    for i in range(NCORES):
        m = dict(consts)
        m["x"] = np.ascontiguousarray(x[i * BL:(i + 1) * BL])
        in_maps.append(m)
    res = run_bass_kernel_spmd(nc, in_maps, list(range(NCORES)), trace=trace)
    tags = np.concatenate(
        [res.results[i]["tags"] for i in range(NCORES)], axis=0)
    return tags.astype(np.int32), res


def kernel(**inputs):
    tags, _ = _run(inputs, trace=False)
    return tags


# revision 29
# speedup vs baseline: 1.0513x; 1.0376x over previous
"""CRF (MLP emissions + Viterbi decode) Trainium2 kernel.

Strategy: data-parallel over batch across 8 NeuronCores (8 sequences per
core).  Inside each core:
  - MLP emissions via PE matmuls (plain fp32 for precision).
  - Viterbi forward pass via a chunk-parallel scan: T=8192 split into
    C=128 chunks of L=64 steps laid out on the 128 SBUF partitions.
    Chunk-boundary scores come from a rank-1 max-plus factorization of the
    per-chunk transition products (one forward and one backward vector scan,
    both vectorized across chunks), then a sequential fold across the 128
    boundaries; within-chunk scores are recomputed with the reference's
    exact fp32 association order.
  - Backpointers are extracted in batches via exact is_equal masks against
    the winning scores (first-index argmax via a reversed-iota/max trick).
  - The backtrace is integer function composition: per-chunk suffix tables,
    per-chunk total maps, a log2 cross-chunk composition (doubling with
    partition-shifted copies), and a final vectorized gather.
"""

import numpy as np

B, T, D, K = 64, 8192, 128, 9
H1, H2 = 128, 64
NCORES = 8
BL = B // NCORES           # batches per core
C, L = 128, 64             # chunks x chunk length (C*L == T)
NBLK = T // 512            # 512-column MLP blocks per batch
NEG = -1.0e30

_CACHE = {}


def _build_program():
    import concourse.bacc as bacc
    import concourse.mybir as mybir
    import concourse.tile as tile

    dt = mybir.dt
    AOP = mybir.AluOpType
    AXX = mybir.AxisListType
    ACTF = mybir.ActivationFunctionType
    f32 = dt.float32
    bf16 = dt.bfloat16

    nc = bacc.Bacc("TRN2", target_bir_lowering=False, debug=False,
                   num_devices=NCORES)

    x_in = nc.dram_tensor("x", [BL, T, D], f32, kind="ExternalInput")
    W1_in = nc.dram_tensor("W1", [D, H1], f32, kind="ExternalInput")
    b1_in = nc.dram_tensor("b1c", [H1, 1], f32, kind="ExternalInput")
    W2_in = nc.dram_tensor("W2", [H1, H2], f32, kind="ExternalInput")
    b2_in = nc.dram_tensor("b2c", [H2, 1], f32, kind="ExternalInput")
    W3e_in = nc.dram_tensor("W3e", [H2 + 1, K], f32, kind="ExternalInput")
    FB = 128 + 2 * K * K + 3 * BL * K + 2 * BL * K
    fblob_in = nc.dram_tensor("fblob", [128, FB], f32, kind="ExternalInput")
    bblob_in = nc.dram_tensor("bblob", [128, 2 * K], bf16, kind="ExternalInput")
    tags_out = nc.dram_tensor("tags", [BL, T], dt.int32, kind="ExternalOutput")

    with tile.TileContext(nc) as tc:
        with tc.tile_pool(name="const", bufs=1) as cp, \
             tc.tile_pool(name="mlp", bufs=3) as mp, \
             tc.tile_pool(name="big", bufs=1) as bg, \
             tc.tile_pool(name="dram", bufs=1, space="DRAM") as dp, \
             tc.tile_pool(name="ps", bufs=2, space="PSUM") as pp:

            # ---- constants ----
            W1 = cp.tile([D, H1], f32, tag="W1")
            b1c = cp.tile([H1, 1], f32, tag="b1c")
            W2 = cp.tile([H1, H2], f32, tag="W2")
            b2c = cp.tile([H2, 1], f32, tag="b2c")
            W3e = cp.tile([H2 + 1, K], f32, tag="W3e")
            fblob = cp.tile([128, FB], f32, tag="fblob")
            bblob = cp.tile([128, 2 * K], bf16, tag="bblob")
            for t_, s_ in ((W1, W1_in), (b1c, b1_in), (W2, W2_in),
                           (b2c, b2_in), (W3e, W3e_in),
                           (fblob, fblob_in), (bblob, bblob_in)):
                nc.sync.dma_start(t_[:], s_[:])
            o = 0
            identf = fblob[:, o:o + 128]; o += 128
            transJI = fblob[:, o:o + K * K].rearrange(
                "p (j i) -> p j i", j=K); o += K * K
            transIK = fblob[:, o:o + K * K].rearrange(
                "p (i k) -> p i k", i=K); o += K * K
            transR0 = fblob[:, o:o + BL * K].rearrange(
                "p (b j) -> p b j", b=BL); o += BL * K
            transC0 = fblob[:, o:o + BL * K].rearrange(
                "p (b j) -> p b j", b=BL); o += BL * K
            endJ = fblob[:, o:o + BL * K].rearrange(
                "p (b j) -> p b j", b=BL); o += BL * K
            startJ = fblob[0:1, o:o + BL * K].rearrange(
                "p (b j) -> p b j", b=BL); o += BL * K
            initf0 = fblob[0:1, o:o + BL * K].rearrange(
                "p (b j) -> p b j", b=BL); o += BL * K
            riotaI = bblob[:, 0:K]
            iotaX = bblob[:, K:2 * K]

            # ---- persistent state ----
            em_sb = bg.tile([128, BL, L, K], f32, tag="em_sb")      # (c|b,l,j)
            h2sA = bg.tile([H2 + 1, 512], f32, tag="h2sA")
            h2sB = bg.tile([H2 + 1, 512], f32, tag="h2sB")
            em_dram = dp.tile([BL, T, K], f32, tag="em_dram")
            scoreH = bg.tile([128, BL, L, K], f32, tag="scoreH")    # (c|b,l,j)
            red1 = bg.tile([128, BL, K], f32, tag="red1")
            rbp = bg.tile([128, BL, L, K], bf16, tag="rbp")
            bpix = bg.tile([128, BL, L, K], bf16, tag="bpix")
            a1b = bg.tile([128, BL, K, K], f32, tag="a1b")
            redb = bg.tile([128, BL, K], f32, tag="redb")
            fstate = bg.tile([128, BL, K], f32, tag="fstate")
            gstate = bg.tile([128, BL, K], f32, tag="gstate")
            ghat = bg.tile([128, BL, K], f32, tag="ghat")
            bounds = bg.tile([128, BL, K], f32, tag="bounds")
            ftil = bg.tile([128, BL, K], f32, tag="ftil")
            fsh = bg.tile([128, BL, K], f32, tag="fsh")
            gsh = bg.tile([128, BL, K], f32, tag="gsh")
            dtile = bg.tile([128, BL, K], f32, tag="dtile")
            dp_ = bg.tile([128, BL], f32, tag="dp_")
            t0b = bg.tile([1, BL, K], f32, tag="t0b")
            a1p = bg.tile([1, BL], f32, tag="a1p")
            dlt_d = dp.tile([128, BL], f32, tag="dlt_d")
            al_d = dp.tile([1, BL], f32, tag="al_d")
            lam_d = dp.tile([BL, 128], f32, tag="lam_d")
            dltB = bg.tile([BL, 128], f32, tag="dltB")
            alB = bg.tile([BL, 1], f32, tag="alB")
            LamB = bg.tile([BL, 128], f32, tag="LamB")
            negB = bg.tile([BL, 128], f32, tag="negB")
            Lamp = bg.tile([128, BL], f32, tag="Lamp")
            Rtab = bg.tile([128, BL, L, K], bf16, tag="Rtab")
            mexp = bg.tile([128, BL, K, K], bf16, tag="mexp")
            gbuf = bg.tile([128, BL, K, K], bf16, tag="gbuf")
            Ztab = bg.tile([128, BL, K], bf16, tag="Ztab")
            Suf = bg.tile([128, BL, K], bf16, tag="Suf")
            Sh = bg.tile([128, BL, K], bf16, tag="Sh")
            rep = bg.tile([128, BL, K], f32, tag="rep")
            ffall = bg.tile([128, BL, K], f32, tag="ffall")
            mxf = bg.tile([128, BL], f32, tag="mxf")
            mke = bg.tile([128, BL, K], bf16, tag="mke")
            mre = bg.tile([128, BL, K], bf16, tag="mre")
            lastrv = bg.tile([128, BL], bf16, tag="lastrv")
            lastix = bg.tile([128, BL], bf16, tag="lastix")
            mk2 = bg.tile([128, BL, K], bf16, tag="mk2")
            mr2 = bg.tile([128, BL, K], bf16, tag="mr2")
            wc = bg.tile([128, BL], bf16, tag="wc")
            mexp4 = bg.tile([128, BL, L, K], bf16, tag="mexp4")
            gb4 = bg.tile([128, BL, L, K], bf16, tag="gb4")
            tagsv = bg.tile([128, BL, L], bf16, tag="tagsv")
            tagsi = bg.tile([128, BL, L], dt.int32, tag="tagsi")

            # =========== MLP: emissions ===========
            # ones row for the b3 fold (h2s partitions 0..63 = relu(h2),
            # partition 64 = 1.0)
            nc.vector.memset(h2sA[64:65], 1.0)
            nc.vector.memset(h2sB[64:65], 1.0)
            for b in range(BL):
                for g in range(2):          # two psum em batches per b
                    emb = pp.tile([128, 32 * K], f32, tag="emb", bufs=1)
                    t0 = g * 4096
                    xt32 = mp.tile([128, 32, 128], f32, tag="xt32", bufs=2)
                    nc.sync.dma_start(
                        xt32[:],
                        x_in[b, t0:t0 + 4096, :].rearrange(
                            "(k t) d -> t k d", k=32))
                    if True:
                        for blk2 in range(8):   # 512-col compute blocks
                            tp = pp.tile([128, 512], f32, tag="tp", bufs=3)
                            for kk in range(4):
                                nc.tensor.transpose(
                                    tp[:, 128 * kk: 128 * (kk + 1)],
                                    xt32[:, 4 * blk2 + kk, :], identf[:])
                            xts = mp.tile([128, 512], f32, tag="xts")
                            nc.scalar.copy(xts[:], tp[:])
                            h1p = pp.tile([128, 512], f32, tag="h1p")
                            nc.tensor.matmul(h1p[:], W1[:], xts[:],
                                             start=True, stop=True)
                            h1s = mp.tile([128, 512], f32, tag="h1s")
                            nc.scalar.activation(h1s[:], h1p[:], ACTF.Relu,
                                                 bias=b1c[:])
                            h2p = pp.tile([64, 512], f32, tag="h2p")
                            nc.tensor.matmul(h2p[:], W2[:], h1s[:],
                                             start=True, stop=True)
                            h2s = h2sA if blk2 % 2 == 0 else h2sB
                            nc.scalar.activation(h2s[0:64], h2p[:], ACTF.Relu,
                                                 bias=b2c[:])
                            # emissions, [t, 9] orientation, b3 via ones row
                            for kk in range(4):
                                m = blk2 * 4 + kk
                                nc.tensor.matmul(
                                    emb[:, K * m: K * (m + 1)],
                                    h2s[0:65, 128 * kk: 128 * (kk + 1)],
                                    W3e[:], start=True, stop=True)
                    # psum em batch -> SBUF staging -> DRAM scratch
                    em_st = mp.tile([128, 32 * K], f32, tag="em_st")
                    nc.scalar.copy(em_st[:], emb[:])
                    nc.sync.dma_start(
                        em_dram[b, g * 4096:(g + 1) * 4096, :].rearrange(
                            "(m t) j -> t m j", m=32),
                        em_st[:].rearrange("t (m j) -> t m j", j=K))
                # whole-batch emissions -> chunk-partitioned SBUF layout
                nc.sync.dma_start(
                    em_sb[:, b],
                    em_dram[b].rearrange("(c l) j -> c l j", c=C))

            def bc(ap, shape):
                return ap.to_broadcast(shape)

            # =========== phase 1: forward + backward scans ===========
            # b-split into halves so the first half's scans overlap the
            # second half's MLP on the (idle) vector engine.
            def phase1_half(b0, b1):
                n = b1 - b0
                fs = fstate[:, b0:b1]
                gs = gstate[:, b0:b1]
                gh = ghat[:, b0:b1]
                ab_ = a1b[:, b0:b1]
                rd = redb[:, b0:b1]
                nc.vector.tensor_tensor(
                    fs, transR0[:, b0:b1], em_sb[:, b0:b1, 0, :], op=AOP.add)
                nc.vector.tensor_copy(fstate[0:1, b0:b1], initf0[:, b0:b1])
                for l in range(1, L):
                    nc.vector.tensor_tensor(
                        ab_,
                        bc(fs.unsqueeze(2), [128, n, K, K]),
                        bc(transJI[:].unsqueeze(1), [128, n, K, K]),
                        op=AOP.add)
                    nc.vector.tensor_reduce(rd, ab_, axis=AXX.X, op=AOP.max)
                    nc.vector.tensor_tensor(
                        fs, rd, em_sb[:, b0:b1, l, :], op=AOP.add)
                nc.vector.tensor_tensor(
                    gs, transC0[:, b0:b1],
                    bc(em_sb[:, b0:b1, L - 1, 0:1], [128, n, K]), op=AOP.add)
                for l in range(L - 2, L - 2 - 15, -1):
                    nc.vector.tensor_tensor(
                        gh, gs, em_sb[:, b0:b1, l, :], op=AOP.add)
                    nc.vector.tensor_tensor(
                        ab_,
                        bc(gh.unsqueeze(2), [128, n, K, K]),
                        bc(transIK[:].unsqueeze(1), [128, n, K, K]),
                        op=AOP.add)
                    nc.vector.tensor_reduce(gs, ab_, axis=AXX.X, op=AOP.max)

            for qq in range(BL):
                phase1_half(qq, qq + 1)

            # =========== phase 2: boundary fold (rank-1 map composition) ====
            # s_c = Lam_c + ftil_{c-1};  Lam via one sequential scan over
            # per-chunk scalars delta_c = max_j(ftil_{c-1} + g_c).
            nc.vector.tensor_tensor(
                ftil[:], fstate[:],
                bc(gstate[:, :, 0:1], [128, BL, K]), op=AOP.subtract)
            nc.sync.dma_start(gsh[0:127], gstate[1:128])
            nc.vector.tensor_tensor(dtile[:], ftil[:], gsh[:], op=AOP.add)
            nc.vector.tensor_reduce(dp_[:], dtile[:], axis=AXX.X, op=AOP.max)
            # alpha_1 = max_j fl(s0 + g_0), s0 into bounds[0]
            nc.vector.tensor_tensor(
                bounds[0:1], startJ, em_sb[0:1, :, 0, :], op=AOP.add)
            nc.vector.tensor_tensor(
                t0b[:], bounds[0:1], gstate[0:1], op=AOP.add)
            nc.vector.tensor_reduce(a1p[:], t0b[:], axis=AXX.X, op=AOP.max)
            # transpose delta/alpha to batch-partition layout
            nc.sync.dma_start(dlt_d[:], dp_[:])
            nc.sync.dma_start(al_d[:], a1p[:])
            nc.sync.dma_start(dltB[:], dlt_d[:].transpose([1, 0]))
            nc.sync.dma_start(alB[:], al_d[:].transpose([1, 0]))
            # Lam scan: LamB[c] = Lam_c; Lam_1 = alpha1; Lam_c += delta
            nc.vector.memset(negB[:], NEG)
            nc.vector.tensor_copy(LamB[:, 1:2], alB[:])
            nc.vector.tensor_tensor_scan(
                LamB[:, 2:128], dltB[:, 0:126], negB[:, 0:126], alB[:],
                op0=AOP.add, op1=AOP.max)
            nc.sync.dma_start(lam_d[:], LamB[:])
            nc.sync.dma_start(Lamp[:], lam_d[:].transpose([1, 0]))
            nc.sync.dma_start(fsh[1:128], ftil[0:127])
            nc.vector.tensor_tensor(
                bounds[:], bc(Lamp[:].unsqueeze(2), [128, BL, K]),
                fsh[:], op=AOP.add)
            nc.vector.tensor_tensor(
                bounds[0:1], startJ, em_sb[0:1, :, 0, :], op=AOP.add)

            # =========== phase 3 + per-step backpointer extraction ===========
            riota_bf4 = bc(riotaI[:].unsqueeze(1).unsqueeze(1),
                           [128, BL, K, K])
            for l in range(L):
                src3 = bounds[:] if l == 0 else scoreH[:, :, l - 1, :]
                nc.vector.tensor_tensor(
                    a1b[:],
                    bc(src3.unsqueeze(2), [128, BL, K, K]),
                    bc(transJI[:].unsqueeze(1), [128, BL, K, K]),
                    op=AOP.add)
                nc.vector.tensor_reduce(
                    red1[:], a1b[:], axis=AXX.X, op=AOP.max)
                nc.vector.tensor_tensor(
                    scoreH[:, :, l, :], red1[:], em_sb[:, :, l, :],
                    op=AOP.add)
                if l == 0:
                    nc.vector.tensor_copy(scoreH[0:1, :, 0, :], bounds[0:1])
                nc.vector.tensor_tensor(
                    mexp[:], a1b[:],
                    bc(red1[:].unsqueeze(3), [128, BL, K, K]),
                    op=AOP.is_equal)
                nc.vector.tensor_tensor(
                    gbuf[:], mexp[:], riota_bf4, op=AOP.mult)
                nc.vector.tensor_reduce(
                    rbp[:, :, l, :], gbuf[:], axis=AXX.X, op=AOP.max)

            # bp index = 9 - rv
            nc.vector.tensor_scalar(
                bpix[:], rbp[:], -1.0, 9.0, op0=AOP.mult, op1=AOP.add)

            # =========== final-step argmax (replicated to all partitions) ====
            nc.sync.dma_start(rep[0:1], scoreH[127:128, :, 63, :])
            n = 1
            while n < 128:
                m = min(n, 128 - n)
                nc.sync.dma_start(rep[n:n + m], rep[0:m])
                n += m
            nc.vector.tensor_tensor(ffall[:], rep[:], endJ[:], op=AOP.add)
            nc.vector.tensor_reduce(mxf[:], ffall[:], axis=AXX.X, op=AOP.max)
            nc.vector.tensor_tensor(
                mke[:], ffall[:], bc(mxf[:].unsqueeze(2), [128, BL, K]),
                op=AOP.is_equal)
            nc.vector.tensor_tensor(
                mre[:], mke[:], bc(riotaI[:].unsqueeze(1), [128, BL, K]),
                op=AOP.mult)
            nc.vector.tensor_reduce(lastrv[:], mre[:], axis=AXX.X, op=AOP.max)
            nc.vector.tensor_scalar(
                lastix[:], lastrv[:], -1.0, 9.0, op0=AOP.mult, op1=AOP.add)

            # =========== backtrace: suffix tables within chunks ===========
            iotaX_b4 = bc(iotaX[:].unsqueeze(1).unsqueeze(1), [128, BL, K, K])
            nc.vector.tensor_copy(
                Rtab[:, :, L - 1, :], bc(iotaX[:].unsqueeze(1), [128, BL, K]))
            for l in range(L - 2, -1, -1):
                nc.vector.tensor_tensor(
                    mexp[:],
                    bc(Rtab[:, :, l + 1, :].unsqueeze(3), [128, BL, K, K]),
                    iotaX_b4, op=AOP.is_equal)
                nc.vector.tensor_tensor(
                    gbuf[:], mexp[:],
                    bc(bpix[:, :, l + 1, :].unsqueeze(2), [128, BL, K, K]),
                    op=AOP.mult)
                nc.vector.tensor_reduce(
                    Rtab[:, :, l, :], gbuf[:], axis=AXX.X, op=AOP.max)

            # per-chunk total map Z_c = bp(c,0) o R_0
            nc.vector.tensor_tensor(
                mexp[:], bc(Rtab[:, :, 0, :].unsqueeze(3), [128, BL, K, K]),
                iotaX_b4, op=AOP.is_equal)
            nc.vector.tensor_tensor(
                gbuf[:], mexp[:],
                bc(bpix[:, :, 0, :].unsqueeze(2), [128, BL, K, K]),
                op=AOP.mult)
            nc.vector.tensor_reduce(Ztab[:], gbuf[:], axis=AXX.X, op=AOP.max)

            # =========== cross-chunk suffix composition (doubling) ===========
            idview = bc(iotaX[:].unsqueeze(1), [128, BL, K])
            nc.vector.tensor_copy(Suf[:], Ztab[:])
            k = 1
            while k < 128:
                nc.vector.tensor_copy(Sh[:], idview)
                nc.sync.dma_start(Sh[0:128 - k], Suf[k:128])
                nc.vector.tensor_tensor(
                    mexp[:], bc(Sh[:].unsqueeze(3), [128, BL, K, K]),
                    iotaX_b4, op=AOP.is_equal)
                nc.vector.tensor_tensor(
                    gbuf[:], mexp[:],
                    bc(Suf[:].unsqueeze(2), [128, BL, K, K]), op=AOP.mult)
                nc.vector.tensor_reduce(Suf[:], gbuf[:], axis=AXX.X, op=AOP.max)
                k *= 2
            # shift by one: Sh_c = Suf_{c+1} (identity at c=127)
            nc.vector.tensor_copy(Sh[:], idview)
            nc.sync.dma_start(Sh[0:127], Suf[1:128])
            # w_c = Sh_c[lastix]
            nc.vector.tensor_tensor(
                mk2[:], idview, bc(lastix[:].unsqueeze(2), [128, BL, K]),
                op=AOP.is_equal)
            nc.vector.tensor_tensor(mr2[:], mk2[:], Sh[:], op=AOP.mult)
            nc.vector.tensor_reduce(wc[:], mr2[:], axis=AXX.X, op=AOP.max)

            # =========== final tag gather ===========
            nc.vector.tensor_tensor(
                mexp4[:],
                bc(iotaX[:].unsqueeze(1).unsqueeze(1), [128, BL, L, K]),
                bc(wc[:].unsqueeze(2).unsqueeze(3), [128, BL, L, K]),
                op=AOP.is_equal)
            nc.vector.tensor_tensor(gb4[:], mexp4[:], Rtab[:], op=AOP.mult)
            nc.vector.tensor_reduce(tagsv[:], gb4[:], axis=AXX.X, op=AOP.max)
            nc.vector.tensor_copy(tagsi[:], tagsv[:])
            nc.sync.dma_start(
                tags_out[:].rearrange("b (c l) -> c b l", c=C), tagsi[:])

    nc.compile()
    return nc


def _consts(W1, b1, W2, b2, W3, b3, start_trans, end_trans, trans):
    import ml_dtypes
    f32 = np.float32
    trans = np.asarray(trans, f32)
    start = np.asarray(start_trans, f32)
    end = np.asarray(end_trans, f32)
    rep128 = lambda a: np.ascontiguousarray(
        np.broadcast_to(a.reshape(1, -1), (128, a.size)).copy())
    parts = [
        np.eye(128, dtype=f32),                                   # identf
        rep128(np.ascontiguousarray(trans.T)),                    # transJI (j,i)
        rep128(np.ascontiguousarray(trans)),                      # transIK (i,k)
        rep128(np.broadcast_to(trans[0][None], (BL, K)).copy()),  # transR0
        rep128(np.broadcast_to(trans[:, 0][None], (BL, K)).copy()),  # transC0
        rep128(np.broadcast_to(end[None], (BL, K)).copy()),       # endJ
    ]
    startJ = np.zeros((128, BL * K), f32)
    startJ[0] = np.broadcast_to(start[None], (BL, K)).reshape(-1)
    initf0 = np.zeros((128, BL * K), f32)
    initf0[0] = np.broadcast_to(
        np.where(np.arange(K) == 0, f32(0), f32(NEG))[None],
        (BL, K)).reshape(-1)
    parts += [startJ, initf0]
    fblob = np.ascontiguousarray(np.concatenate(parts, axis=1), f32)
    bblob = np.concatenate(
        [rep128(9.0 - np.arange(K, dtype=f32)),
         rep128(np.arange(K, dtype=f32))], axis=1).astype(ml_dtypes.bfloat16)
    return {
        "W1": np.ascontiguousarray(W1, f32),
        "b1c": np.ascontiguousarray(np.asarray(b1, f32).reshape(H1, 1)),
        "W2": np.ascontiguousarray(W2, f32),
        "b2c": np.ascontiguousarray(np.asarray(b2, f32).reshape(H2, 1)),
        "W3e": np.ascontiguousarray(
            np.concatenate([np.asarray(W3, f32),
                            np.asarray(b3, f32).reshape(1, K)], axis=0)),
        "fblob": fblob, "bblob": np.ascontiguousarray(bblob),
    }


def _run(inputs, trace=False):
    from concourse.bass_utils import run_bass_kernel_spmd
    if "nc" not in _CACHE:
        _CACHE["nc"] = _build_program()
    nc = _CACHE["nc"]
    x = np.asarray(inputs["x"], np.float32)
    consts = _consts(
        inputs["W1"], inputs["b1"], inputs["W2"], inputs["b2"],
        inputs["W3"], inputs["b3"], inputs["start_trans"],
        inputs["end_trans"], inputs["trans"])
    in_maps = []
    for i in range(NCORES):
        m = dict(consts)
        m["x"] = np.ascontiguousarray(x[i * BL:(i + 1) * BL])
        in_maps.append(m)
    res = run_bass_kernel_spmd(nc, in_maps, list(range(NCORES)), trace=trace)
    tags = np.concatenate(
        [res.results[i]["tags"] for i in range(NCORES)], axis=0)
    return tags.astype(np.int32), res


def kernel(**inputs):
    tags, _ = _run(inputs, trace=False)
    return tags
